# revision 8
# baseline (speedup 1.0000x reference)
"""DYSPN attention-conv kernel for Trainium2 (8 NeuronCores, batch-parallel).

Math (unfold/fold pair collapses algebraically; att >= 0):
  per image, per tap k=(i,j) != center, ring r = INDEX[i,j], dy = 3-i, dx = 3-j:
    z_k[y,x] = att_r[y,x] * aff_k[y,x]
    U[y,x]   = sum_k z_k[y,x]                      (S_ppt - att3)
    A[y,x]   = sum_k |z_k[y,x]|                    (S_prime - att3)
    T[y,x]   = sum_k z_k[y+dy, x+dx]  (in-image)   (fold7(z))
  out = ((T+att3)*cs - (U+att3)*co) / (A+att3+eps) + co

Layout: batch 16 -> 2 images/core. Row-parity twin tiles: partition p holds
image row 2p in the "A" tile and row 2p+1 in the "B" tile, so a full 256-row
image fits the 128 partitions and fold's row shifts never cross a block seam:
  dy even (=2m):  outS[p] += zS[p+m]        (same-parity tile, band offset m)
  dy odd:         outA[p] += zB[p+(dy-1)/2], outB[p] += zA[p+(dy+1)/2]
Row shifts = band-matrix lhsT offsets; col shifts = moving-side offsets into
GW-guarded planes (guards zeroed once; DMA/mult only touch them with 0*att).

Engine split (per-core busy estimates vs ~85us DMA roofline):
  PE   ~73us: U/A/T reductions as banded fp32r/bf16 matmuls into PSUM
  DVE  ~65us: z = att*aff (in-place, guards included), reciprocal
  ACT  ~55us: |z| -> bf16, PSUM->SBUF epilogue staging
  Pool ~23us: z-mult for the i=5 tap row, SBUF epilogue math, guard memsets
  DMA: graded chunks, small head (fast PE start) and tail (short drain).
"""
import sys

sys.path.insert(0, "/opt/trn_rl_repo")

import numpy as np

import concourse.bass as bass  # noqa: F401  (registers engines)
import concourse.tile as tile
from concourse import bacc, mybir
from concourse.bass_utils import run_bass_kernel_spmd

FP32 = mybir.dt.float32
FP32R = mybir.dt.float32r
BF16 = mybir.dt.bfloat16
U32 = mybir.dt.uint32

N_CORES = 8
B_FULL = 16
B_CORE = B_FULL // N_CORES  # 2 images per core
H = W = 256
K = 7
GW = 4                    # zero guard columns each side of a z/att plane
WG = W + 2 * GW           # guarded plane width (264)
C0 = 2                    # band diagonal column offset (shifts in [-2, 2])
BANDW = 132
EPS = 1e-6

# graded chunk boundaries in t-space (48 taps, center excluded); chunk
# [34,41) is exactly the i=5 row (handled by GpSimd), tail chunks small
CHUNKS = [(0, 2), (2, 6), (6, 12), (12, 18), (18, 24), (24, 30), (30, 34),
          (34, 41), (41, 44), (44, 46), (46, 48)]
POOL_CHUNK = 7            # chunk index whose z-mult runs on GpSimd
MAXT = max(hi - lo for lo, hi in CHUNKS)  # 7
NZBUF = 6                 # z ring depth (per parity)
NABUF = 3                 # |z| ring depth (per parity)

# ring index of each tap in the 7x7 window (center marked 3, excluded)
_INDEX = np.array([0, 0, 0, 0, 0, 0, 0,
                   0, 1, 1, 1, 1, 1, 0,
                   0, 1, 2, 2, 2, 1, 0,
                   0, 1, 2, 3, 2, 1, 0,
                   0, 1, 2, 2, 2, 1, 0,
                   0, 1, 1, 1, 1, 1, 0,
                   0, 0, 0, 0, 0, 0, 0], dtype=np.int64).reshape(7, 7)

TAPS = []  # (t, k, ring, dy, dx)
for i in range(K):
    for j in range(K):
        if i == 3 and j == 3:
            continue
        k = i * K + j
        t = k if k < 24 else k - 1
        TAPS.append((t, k, int(_INDEX[i, j]), 3 - i, 3 - j))

# maximal same-ring runs in t-order (for the z-mult broadcast)
RUNS = []  # [t_lo, t_hi, ring]
for t, k, r, dy, dx in TAPS:
    if RUNS and RUNS[-1][2] == r and RUNS[-1][1] == t:
        RUNS[-1][1] = t + 1
    else:
        RUNS.append([t, t + 1, r])


def _band_matrix() -> np.ndarray:
    band = np.zeros((128, BANDW), dtype=np.float32)
    for p in range(128):
        band[p, p + C0] = 1.0
    return band


def _build():
    nc = bacc.Bacc("TRN2", target_bir_lowering=False, debug=False,
                   num_devices=N_CORES)
    aff = nc.dram_tensor("affinity", [B_CORE, 49, H, W], FP32R,
                         kind="ExternalInput").ap()
    att = nc.dram_tensor("attention", [B_CORE, 4, H, W], FP32,
                         kind="ExternalInput").ap()
    cs = nc.dram_tensor("current_segmentation", [B_CORE, 1, H, W], FP32,
                        kind="ExternalInput").ap()
    co = nc.dram_tensor("coarse_segmentation", [B_CORE, 1, H, W], FP32,
                        kind="ExternalInput").ap()
    band = nc.dram_tensor("band", [128, BANDW], FP32, kind="ExternalInput").ap()
    out = nc.dram_tensor("out", [B_CORE, 1, H, W], FP32,
                         kind="ExternalOutput").ap()

    # DRAM views with rows split by parity: [.., p, s, x]
    aff_ps = aff.rearrange("b k (p s) x -> b k p s x", s=2)
    att_ps = att.rearrange("b r (p s) x -> b r p s x", s=2)
    cs_ps = cs.rearrange("b o (p s) x -> b o p s x", s=2)
    co_ps = co.rearrange("b o (p s) x -> b o p s x", s=2)
    out_ps = out.rearrange("b o (p s) x -> b o p s x", s=2)

    with tile.TileContext(nc) as tc:
        with tc.tile_pool(name="sb", bufs=1) as sb, \
             tc.tile_pool(name="ps", bufs=1, space="PSUM") as ps:

            bandf = sb.tile([128, BANDW], FP32, tag="bandf")
            nc.sync.dma_start(out=bandf[:], in_=band[:, :])
            bandr = sb.tile([128, BANDW], FP32R, tag="bandr")
            nc.vector.tensor_copy(bandr[:], bandf[:])
            identr = bandr[:, C0:C0 + 128]
            identb = sb.tile([128, 128], BF16, tag="identb")
            nc.vector.tensor_copy(identb[:], bandf[:, C0:C0 + 128])

            # persistent rings; guards zeroed once, never rewritten
            zt = [[sb.tile([128, MAXT, WG], FP32R, tag=f"zt{s}_{b}", name=f"zt{s}_{b}")
                   for b in range(NZBUF)] for s in range(2)]
            azt = [[sb.tile([128, MAXT, WG], BF16, tag=f"az{s}_{b}", name=f"az{s}_{b}")
                    for b in range(NABUF)] for s in range(2)]
            for s in range(2):
                for zb in zt[s]:
                    nc.gpsimd.memset(zb[:, :, 0:GW].bitcast(U32), 0)
                    nc.gpsimd.memset(zb[:, :, GW + W:].bitcast(U32), 0)

            attf = [[sb.tile([128, 4, WG], FP32, tag=f"attf{s}_{img}", name=f"attf{s}_{img}")
                     for s in range(2)] for img in range(B_CORE)]
            for img in range(B_CORE):
                for s in range(2):
                    nc.gpsimd.memset(attf[img][s][:, :, 0:GW].bitcast(U32), 0)
                    nc.gpsimd.memset(attf[img][s][:, :, GW + W:].bitcast(U32), 0)

            att3r = [[sb.tile([128, W], FP32R, tag=f"att3r{s}_{img}",
                              name=f"att3r{s}_{img}")
                      for s in range(2)] for img in range(B_CORE)]
            cst = [[sb.tile([128, W], FP32, tag=f"cst{s}_{img}", name=f"cst{s}_{img}")
                    for s in range(2)] for img in range(B_CORE)]
            cot = [[sb.tile([128, W], FP32, tag=f"cot{s}_{img}", name=f"cot{s}_{img}")
                    for s in range(2)] for img in range(B_CORE)]

            psU = [ps.tile([128, 2, W], FP32, tag=f"U{img}", name=f"psU{img}")
                   for img in range(B_CORE)]
            psA = [ps.tile([128, 2, W], FP32, tag=f"A{img}", name=f"psA{img}")
                   for img in range(B_CORE)]
            psT = [ps.tile([128, 2, W], FP32, tag=f"T{img}", name=f"psT{img}")
                   for img in range(B_CORE)]

            # epilogue staging (SBUF)
            uT = [[sb.tile([128, W], FP32, tag=f"uT{s}_{img}", name=f"uT{s}_{img}") for s in range(2)]
                  for img in range(B_CORE)]
            uU = [[sb.tile([128, W], FP32, tag=f"uU{s}_{img}", name=f"uU{s}_{img}") for s in range(2)]
                  for img in range(B_CORE)]
            uA = [[sb.tile([128, W], FP32, tag=f"uA{s}_{img}", name=f"uA{s}_{img}") for s in range(2)]
                  for img in range(B_CORE)]
            rcp = [[sb.tile([128, W], FP32, tag=f"rcp{s}_{img}", name=f"rcp{s}_{img}") for s in range(2)]
                   for img in range(B_CORE)]
            m1 = [[sb.tile([128, W], FP32, tag=f"m1{s}_{img}", name=f"m1{s}_{img}") for s in range(2)]
                  for img in range(B_CORE)]
            m2 = [[sb.tile([128, W], FP32, tag=f"m2{s}_{img}", name=f"m2{s}_{img}") for s in range(2)]
                  for img in range(B_CORE)]

            def small_inputs(img):
                for s in range(2):
                    nc.sync.dma_start(
                        out=attf[img][s][:, :, GW:GW + W],
                        in_=att_ps[img, :, :, s, :].transpose([1, 0, 2]))
                    nc.scalar.activation(att3r[img][s][:],
                                         attf[img][s][:, 3, GW:GW + W],
                                         mybir.ActivationFunctionType.Copy)
                    nc.sync.dma_start(out=cst[img][s][:],
                                      in_=cs_ps[img, 0, :, s, :])
                    nc.sync.dma_start(out=cot[img][s][:],
                                      in_=co_ps[img, 0, :, s, :])

            def chunk_dma(img, ci):
                lo, hi = CHUNKS[ci]
                k_lo = lo if lo < 24 else lo + 1
                for s in range(2):
                    nc.sync.dma_start(
                        out=zt[s][ci % NZBUF][:, 0:hi - lo, GW:GW + W],
                        in_=aff_ps[img, k_lo:k_lo + hi - lo, :, s, :]
                            .transpose([1, 0, 2]))

            def chunk_mult(img, ci):
                lo, hi = CHUNKS[ci]
                eng = nc.gpsimd if ci == POOL_CHUNK else nc.vector
                for s in range(2):
                    zb = zt[s][ci % NZBUF]
                    for rlo, rhi, r in RUNS:
                        a, b = max(rlo, lo), min(rhi, hi)
                        if a >= b:
                            continue
                        zs = zb[:, a - lo:b - lo, :]
                        eng.tensor_tensor(
                            out=zs,
                            in0=zs.bitcast(FP32),
                            in1=attf[img][s][:, r:r + 1, :]
                                .broadcast_to([128, b - a, WG]),
                            op=mybir.AluOpType.mult)

            def chunk_abs(img, ci):
                lo, hi = CHUNKS[ci]
                for s in range(2):
                    nc.scalar.activation(
                        azt[s][ci % NABUF][:, 0:hi - lo, :],
                        zt[s][ci % NZBUF][:, 0:hi - lo, :],
                        mybir.ActivationFunctionType.Abs)

            def chunk_ut(img, ci, first):
                lo, hi = CHUNKS[ci]
                for s in range(2):
                    zb = zt[s][ci % NZBUF]
                    for t, k, r, dy, dx in TAPS[lo:hi]:
                        nc.tensor.matmul(
                            out=psU[img][:, s, :], lhsT=identr,
                            rhs=zb[:, t - lo, GW:GW + W],
                            start=(first and s == 0 and t == lo), stop=False,
                            skip_group_check=True)
                # T: band row-shift + guarded col-shift
                for t, k, r, dy, dx in TAPS[lo:hi]:
                    for s in range(2):
                        zb = zt[s][ci % NZBUF]
                        rhs = zb[:, t - lo, GW + dx:GW + dx + W]
                        if dy % 2 == 0:
                            m, so = dy // 2, s
                        elif s == 1:       # reads B-parity, writes A-parity
                            m, so = (dy - 1) // 2, 0
                        else:              # reads A-parity, writes B-parity
                            m, so = (dy + 1) // 2, 1
                        nc.tensor.matmul(
                            out=psT[img][:, so, :],
                            lhsT=bandr[:, C0 + m:C0 + m + 128], rhs=rhs,
                            start=(first and t == lo and s == 0), stop=False,
                            skip_group_check=True)

            def chunk_a(img, ci, first):
                lo, hi = CHUNKS[ci]
                for s in range(2):
                    ab = azt[s][ci % NABUF]
                    for t, k, r, dy, dx in TAPS[lo:hi]:
                        nc.tensor.matmul(
                            out=psA[img][:, s, :], lhsT=identb,
                            rhs=ab[:, t - lo, GW:GW + W],
                            start=(first and s == 0 and t == lo), stop=False,
                            skip_group_check=True)

            def closers(img):
                for s in range(2):
                    a3 = att3r[img][s][:]
                    nc.tensor.matmul(out=psU[img][:, s, :], lhsT=identr,
                                     rhs=a3, start=False, stop=(s == 1),
                                     skip_group_check=True)
                    nc.tensor.matmul(out=psA[img][:, s, :], lhsT=identr,
                                     rhs=a3, start=False, stop=(s == 1),
                                     skip_group_check=True)
                    nc.tensor.matmul(out=psT[img][:, s, :], lhsT=identr,
                                     rhs=a3, start=False, stop=(s == 1),
                                     skip_group_check=True)

            def epilogue(img):
                for s in range(2):
                    # DVE: PSUM -> SBUF staging (+eps on the A path), recip
                    nc.vector.tensor_scalar_add(uA[img][s][:], psA[img][:, s, :],
                                                EPS)
                    nc.vector.tensor_copy(uT[img][s][:], psT[img][:, s, :])
                    nc.vector.tensor_copy(uU[img][s][:], psU[img][:, s, :])
                    nc.vector.reciprocal(rcp[img][s][:], uA[img][s][:])
                    # Pool: SBUF-side chain
                    g = nc.gpsimd
                    g.tensor_mul(m1[img][s][:], uT[img][s][:], cst[img][s][:])
                    g.tensor_mul(m2[img][s][:], uU[img][s][:], cot[img][s][:])
                    g.tensor_sub(m1[img][s][:], m1[img][s][:], m2[img][s][:])
                    g.tensor_mul(m1[img][s][:], m1[img][s][:], rcp[img][s][:])
                    g.tensor_add(m1[img][s][:], m1[img][s][:], cot[img][s][:])
                    nc.scalar.dma_start(out=out_ps[img, 0, :, s, :],
                                        in_=m1[img][s][:])

            # ---- schedule ----
            # Image 0's epilogue is injected into image 1's chunk loop so no
            # engine's in-order stream blocks on image-0 PE completion while
            # image-1 data is already flowing.
            nchunks = len(CHUNKS)
            small_inputs(0)
            for img in range(B_CORE):
                for ci in range(nchunks):
                    chunk_dma(img, ci)
                    if img == 0 and ci == nchunks - 2:
                        small_inputs(1)
                    chunk_mult(img, ci)
                    chunk_abs(img, ci)
                    chunk_ut(img, ci, first=(ci == 0))
                    # defer A matmuls one chunk so PE never waits on ACT
                    if ci > 0:
                        chunk_a(img, ci - 1, first=(ci == 1))
                    if img == 1 and ci == 5:
                        epilogue(0)
                chunk_a(img, nchunks - 1, first=False)
                closers(img)
            epilogue(1)

    nc.compile()
    return nc


_NC_CACHE = None


def _get_nc():
    global _NC_CACHE
    if _NC_CACHE is None:
        _NC_CACHE = _build()
    return _NC_CACHE


def run(inputs: dict, trace: bool = False):
    """Run on 8 NeuronCores; returns (out [16,1,256,256], BassKernelResults)."""
    aff = np.ascontiguousarray(np.asarray(inputs["affinity"], dtype=np.float32))
    att = np.ascontiguousarray(np.asarray(inputs["attention"], dtype=np.float32))
    cs = np.ascontiguousarray(
        np.asarray(inputs["current_segmentation"], dtype=np.float32))
    co = np.ascontiguousarray(
        np.asarray(inputs["coarse_segmentation"], dtype=np.float32))
    band = _band_matrix()

    nc = _get_nc()
    in_maps = []
    for c in range(N_CORES):
        s = slice(c * B_CORE, (c + 1) * B_CORE)
        in_maps.append({
            "affinity": np.ascontiguousarray(aff[s]),
            "attention": np.ascontiguousarray(att[s]),
            "current_segmentation": np.ascontiguousarray(cs[s]),
            "coarse_segmentation": np.ascontiguousarray(co[s]),
            "band": band,
        })
    last_err = None
    for attempt in range(3):
        try:
            res = run_bass_kernel_spmd(nc, in_maps, list(range(N_CORES)),
                                       trace=trace)
            break
        except Exception as e:  # transient NRT_EXEC_UNIT_UNRECOVERABLE flakes
            last_err = e
            import time
            time.sleep(10)
    else:
        raise last_err
    full = np.concatenate([res.results[c]["out"] for c in range(N_CORES)], axis=0)
    return full, res


def kernel(**inputs) -> np.ndarray:
    out, _ = run(inputs, trace=False)
    return out


# revision 10
# speedup vs baseline: 1.1070x; 1.1070x over previous
"""DYSPN attention-conv kernel for Trainium2 (8 NeuronCores, batch-parallel).

Math (unfold/fold pair collapses algebraically; att >= 0):
  per image, per tap k=(i,j) != center, ring r = INDEX[i,j], dy = 3-i, dx = 3-j:
    z_k[y,x] = att_r[y,x] * aff_k[y,x]
    U[y,x]   = sum_k z_k[y,x]                      (S_ppt - att3)
    A[y,x]   = sum_k |z_k[y,x]|                    (S_prime - att3)
    T[y,x]   = sum_k z_k[y+dy, x+dx]  (in-image)   (fold7(z))
  out = ((T+att3)*cs - (U+att3)*co) / (A+att3+eps) + co

Layout: batch 16 -> 2 images/core. Row-parity twin tiles: partition p holds
image row 2p in the "A" tile and row 2p+1 in the "B" tile, so a full 256-row
image fits the 128 partitions and fold's row shifts never cross a block seam:
  dy even (=2m):  outS[p] += zS[p+m]        (same-parity tile, band offset m)
  dy odd:         outA[p] += zB[p+(dy-1)/2], outB[p] += zA[p+(dy+1)/2]
Row shifts = band-matrix lhsT offsets; col shifts = moving-side offsets into
GW-guarded planes (guards zeroed once; DMA/mult only touch them with 0*att).

Engine split (per-core busy estimates vs ~85us DMA roofline):
  PE   ~73us: U/A/T reductions as banded fp32r/bf16 matmuls into PSUM
  DVE  ~65us: z = att*aff (in-place, guards included), reciprocal
  ACT  ~55us: |z| -> bf16, PSUM->SBUF epilogue staging
  Pool ~23us: z-mult for the i=5 tap row, SBUF epilogue math, guard memsets
  DMA: graded chunks, small head (fast PE start) and tail (short drain).
"""
import sys

sys.path.insert(0, "/opt/trn_rl_repo")

import numpy as np

import concourse.bass as bass  # noqa: F401  (registers engines)
import concourse.tile as tile
from concourse import bacc, mybir
from concourse.bass_utils import run_bass_kernel_spmd

FP32 = mybir.dt.float32
FP32R = mybir.dt.float32r
BF16 = mybir.dt.bfloat16
U32 = mybir.dt.uint32

N_CORES = 8
B_FULL = 16
B_CORE = B_FULL // N_CORES  # 2 images per core
H = W = 256
K = 7
GW = 4                    # zero guard columns each side of a z/att plane
WG = W + 2 * GW           # guarded plane width (264)
C0 = 2                    # band diagonal column offset (shifts in [-2, 2])
BANDW = 132
EPS = 1e-6

# graded chunk boundaries in t-space (48 taps, center excluded); chunk
# [34,41) is exactly the i=5 row (handled by GpSimd), tail chunks small
CHUNKS = [(0, 2), (2, 6), (6, 12), (12, 18), (18, 24), (24, 30), (30, 34),
          (34, 41), (41, 44), (44, 46), (46, 48)]
MAXT = max(hi - lo for lo, hi in CHUNKS)  # 7
NZBUF = 7                 # z ring depth (per parity)
NABUF = 3                 # |z| ring depth (per parity)

# ring index of each tap in the 7x7 window (center marked 3, excluded)
_INDEX = np.array([0, 0, 0, 0, 0, 0, 0,
                   0, 1, 1, 1, 1, 1, 0,
                   0, 1, 2, 2, 2, 1, 0,
                   0, 1, 2, 3, 2, 1, 0,
                   0, 1, 2, 2, 2, 1, 0,
                   0, 1, 1, 1, 1, 1, 0,
                   0, 0, 0, 0, 0, 0, 0], dtype=np.int64).reshape(7, 7)

TAPS = []  # (t, k, ring, dy, dx)
for i in range(K):
    for j in range(K):
        if i == 3 and j == 3:
            continue
        k = i * K + j
        t = k if k < 24 else k - 1
        TAPS.append((t, k, int(_INDEX[i, j]), 3 - i, 3 - j))

# maximal same-ring runs in t-order (for the z-mult broadcast)
RUNS = []  # [t_lo, t_hi, ring]
for t, k, r, dy, dx in TAPS:
    if RUNS and RUNS[-1][2] == r and RUNS[-1][1] == t:
        RUNS[-1][1] = t + 1
    else:
        RUNS.append([t, t + 1, r])


def _band_matrix() -> np.ndarray:
    band = np.zeros((128, BANDW), dtype=np.float32)
    for p in range(128):
        band[p, p + C0] = 1.0
    return band


def _build():
    nc = bacc.Bacc("TRN2", target_bir_lowering=False, debug=False,
                   num_devices=N_CORES)
    aff = nc.dram_tensor("affinity", [B_CORE, 49, H, W], FP32R,
                         kind="ExternalInput").ap()
    att = nc.dram_tensor("attention", [B_CORE, 4, H, W], FP32,
                         kind="ExternalInput").ap()
    cs = nc.dram_tensor("current_segmentation", [B_CORE, 1, H, W], FP32,
                        kind="ExternalInput").ap()
    co = nc.dram_tensor("coarse_segmentation", [B_CORE, 1, H, W], FP32,
                        kind="ExternalInput").ap()
    band = nc.dram_tensor("band", [128, BANDW], FP32, kind="ExternalInput").ap()
    out = nc.dram_tensor("out", [B_CORE, 1, H, W], FP32,
                         kind="ExternalOutput").ap()

    # DRAM views with rows split by parity: [.., p, s, x]
    aff_ps = aff.rearrange("b k (p s) x -> b k p s x", s=2)
    att_ps = att.rearrange("b r (p s) x -> b r p s x", s=2)
    cs_ps = cs.rearrange("b o (p s) x -> b o p s x", s=2)
    co_ps = co.rearrange("b o (p s) x -> b o p s x", s=2)
    out_ps = out.rearrange("b o (p s) x -> b o p s x", s=2)

    with tile.TileContext(nc) as tc:
        with tc.tile_pool(name="sb", bufs=1) as sb, \
             tc.tile_pool(name="ps", bufs=1, space="PSUM") as ps:

            bandf = sb.tile([128, BANDW], FP32, tag="bandf")
            nc.sync.dma_start(out=bandf[:], in_=band[:, :])
            bandr = sb.tile([128, BANDW], FP32R, tag="bandr")
            nc.vector.tensor_copy(bandr[:], bandf[:])
            identr = bandr[:, C0:C0 + 128]
            identb = sb.tile([128, 128], BF16, tag="identb")
            nc.vector.tensor_copy(identb[:], bandf[:, C0:C0 + 128])

            # persistent rings; guards zeroed once, never rewritten
            zt = [[sb.tile([128, MAXT, WG], FP32R, tag=f"zt{s}_{b}", name=f"zt{s}_{b}")
                   for b in range(NZBUF)] for s in range(2)]
            azt = [[sb.tile([128, MAXT, WG], BF16, tag=f"az{s}_{b}", name=f"az{s}_{b}")
                    for b in range(NABUF)] for s in range(2)]
            for s in range(2):
                for zb in zt[s]:
                    nc.vector.memset(zb[:, :, 0:GW].bitcast(U32), 0)
                    nc.vector.memset(zb[:, :, GW + W:].bitcast(U32), 0)

            attf = [[sb.tile([128, 4, WG], FP32, tag=f"attf{s}_{img}", name=f"attf{s}_{img}")
                     for s in range(2)] for img in range(B_CORE)]
            att3r = [[sb.tile([128, W], FP32R, tag=f"att3r{s}_{img}",
                              name=f"att3r{s}_{img}")
                      for s in range(2)] for img in range(B_CORE)]
            att3e = [[sb.tile([128, W], FP32R, tag=f"att3e{s}_{img}",
                              name=f"att3e{s}_{img}")
                      for s in range(2)] for img in range(B_CORE)]
            cst = [[sb.tile([128, W], FP32, tag=f"cst{s}_{img}", name=f"cst{s}_{img}")
                    for s in range(2)] for img in range(B_CORE)]
            cot = [[sb.tile([128, W], FP32, tag=f"cot{s}_{img}", name=f"cot{s}_{img}")
                    for s in range(2)] for img in range(B_CORE)]

            psU = [ps.tile([128, 2, W], FP32, tag=f"U{img}", name=f"psU{img}")
                   for img in range(B_CORE)]
            psA = [ps.tile([128, 2, W], FP32, tag=f"A{img}", name=f"psA{img}")
                   for img in range(B_CORE)]
            psT = [ps.tile([128, 2, W], FP32, tag=f"T{img}", name=f"psT{img}")
                   for img in range(B_CORE)]

            # epilogue staging (SBUF)
            uT = [[sb.tile([128, W], FP32, tag=f"uT{s}_{img}", name=f"uT{s}_{img}") for s in range(2)]
                  for img in range(B_CORE)]
            uU = [[sb.tile([128, W], FP32, tag=f"uU{s}_{img}", name=f"uU{s}_{img}") for s in range(2)]
                  for img in range(B_CORE)]
            rcp = [[sb.tile([128, W], FP32, tag=f"rcp{s}_{img}", name=f"rcp{s}_{img}") for s in range(2)]
                   for img in range(B_CORE)]
            m1 = [[sb.tile([128, W], FP32, tag=f"m1{s}_{img}", name=f"m1{s}_{img}") for s in range(2)]
                  for img in range(B_CORE)]
            m2 = [[sb.tile([128, W], FP32, tag=f"m2{s}_{img}", name=f"m2{s}_{img}") for s in range(2)]
                  for img in range(B_CORE)]

            def att_dma(img):
                for s in range(2):
                    nc.sync.dma_start(
                        out=attf[img][s][:, :, GW:GW + W],
                        in_=att_ps[img, :, :, s, :].transpose([1, 0, 2]))

            def att3r_copy(img):
                for s in range(2):
                    nc.scalar.activation(att3r[img][s][:],
                                         attf[img][s][:, 3, GW:GW + W],
                                         mybir.ActivationFunctionType.Copy)
                    # A-path closer carries att3 + eps (Copy keeps float bias)
                    nc.scalar.activation(att3e[img][s][:],
                                         attf[img][s][:, 3, GW:GW + W],
                                         mybir.ActivationFunctionType.Copy,
                                         bias=EPS)

            def seg_dma(img):
                for s in range(2):
                    nc.sync.dma_start(out=cst[img][s][:],
                                      in_=cs_ps[img, 0, :, s, :])
                    nc.sync.dma_start(out=cot[img][s][:],
                                      in_=co_ps[img, 0, :, s, :])

            def chunk_dma(img, ci):
                lo, hi = CHUNKS[ci]
                k_lo = lo if lo < 24 else lo + 1
                for s in range(2):
                    nc.sync.dma_start(
                        out=zt[s][ci % NZBUF][:, 0:hi - lo, GW:GW + W],
                        in_=aff_ps[img, k_lo:k_lo + hi - lo, :, s, :]
                            .transpose([1, 0, 2]))

            def chunk_mult(img, ci):
                lo, hi = CHUNKS[ci]
                for s in range(2):
                    zb = zt[s][ci % NZBUF]
                    for rlo, rhi, r in RUNS:
                        a, b = max(rlo, lo), min(rhi, hi)
                        if a >= b:
                            continue
                        zs = zb[:, a - lo:b - lo, GW:GW + W]
                        nc.vector.tensor_tensor(
                            out=zs,
                            in0=zs.bitcast(FP32),
                            in1=attf[img][s][:, r:r + 1, GW:GW + W]
                                .broadcast_to([128, b - a, W]),
                            op=mybir.AluOpType.mult)

            def chunk_abs(img, ci):
                lo, hi = CHUNKS[ci]
                for s in range(2):
                    nc.scalar.activation(
                        azt[s][ci % NABUF][:, 0:hi - lo, GW:GW + W],
                        zt[s][ci % NZBUF][:, 0:hi - lo, GW:GW + W],
                        mybir.ActivationFunctionType.Abs)

            def chunk_ut(img, ci, first):
                lo, hi = CHUNKS[ci]
                for s in range(2):
                    zb = zt[s][ci % NZBUF]
                    for t, k, r, dy, dx in TAPS[lo:hi]:
                        nc.tensor.matmul(
                            out=psU[img][:, s, :], lhsT=identr,
                            rhs=zb[:, t - lo, GW:GW + W],
                            start=(first and s == 0 and t == lo), stop=False,
                            skip_group_check=True)
                # T: band row-shift + guarded col-shift
                for t, k, r, dy, dx in TAPS[lo:hi]:
                    for s in range(2):
                        zb = zt[s][ci % NZBUF]
                        rhs = zb[:, t - lo, GW + dx:GW + dx + W]
                        if dy % 2 == 0:
                            m, so = dy // 2, s
                        elif s == 1:       # reads B-parity, writes A-parity
                            m, so = (dy - 1) // 2, 0
                        else:              # reads A-parity, writes B-parity
                            m, so = (dy + 1) // 2, 1
                        nc.tensor.matmul(
                            out=psT[img][:, so, :],
                            lhsT=bandr[:, C0 + m:C0 + m + 128], rhs=rhs,
                            start=(first and t == lo and s == 0), stop=False,
                            skip_group_check=True)

            def chunk_a(img, ci, first):
                lo, hi = CHUNKS[ci]
                for s in range(2):
                    ab = azt[s][ci % NABUF]
                    for t, k, r, dy, dx in TAPS[lo:hi]:
                        nc.tensor.matmul(
                            out=psA[img][:, s, :], lhsT=identb,
                            rhs=ab[:, t - lo, GW:GW + W],
                            start=(first and s == 0 and t == lo), stop=False,
                            skip_group_check=True)

            def closers(img):
                for s in range(2):
                    a3 = att3r[img][s][:]
                    nc.tensor.matmul(out=psU[img][:, s, :], lhsT=identr,
                                     rhs=a3, start=False, stop=(s == 1),
                                     skip_group_check=True)
                    nc.tensor.matmul(out=psA[img][:, s, :], lhsT=identr,
                                     rhs=att3e[img][s][:], start=False,
                                     stop=(s == 1), skip_group_check=True)
                    nc.tensor.matmul(out=psT[img][:, s, :], lhsT=identr,
                                     rhs=a3, start=False, stop=(s == 1),
                                     skip_group_check=True)

            def epilogue(img):
                for s in range(2):
                    # ACT: PSUM -> SBUF staging; DVE: fast 1/(A+att3+eps)
                    nc.scalar.activation(uT[img][s][:], psT[img][:, s, :],
                                         mybir.ActivationFunctionType.Copy)
                    nc.scalar.activation(uU[img][s][:], psU[img][:, s, :],
                                         mybir.ActivationFunctionType.Copy)
                    nc.vector.reciprocal_approx_fast(rcp[img][s][:],
                                                     psA[img][:, s, :])
                    # Pool: numerator (independent of recip)
                    g = nc.gpsimd
                    g.tensor_mul(m1[img][s][:], uT[img][s][:], cst[img][s][:])
                    g.tensor_mul(m2[img][s][:], uU[img][s][:], cot[img][s][:])
                    g.tensor_sub(m1[img][s][:], m1[img][s][:], m2[img][s][:])
                    # DVE: apply recip + coarse add (short tail chain)
                    nc.vector.tensor_mul(m1[img][s][:], m1[img][s][:],
                                         rcp[img][s][:])
                    nc.vector.tensor_add(m1[img][s][:], m1[img][s][:],
                                         cot[img][s][:])
                    nc.scalar.dma_start(out=out_ps[img, 0, :, s, :],
                                        in_=m1[img][s][:])

            # ---- schedule ----
            # Image 0's epilogue is injected into image 1's chunk loop so no
            # engine's in-order stream blocks on image-0 PE completion while
            # image-1 data is already flowing.
            nchunks = len(CHUNKS)
            att_dma(0)
            att3r_copy(0)
            for img in range(B_CORE):
                for ci in range(nchunks):
                    chunk_dma(img, ci)
                    if img == 0 and ci == 2:
                        seg_dma(0)
                    if img == 0 and ci == 6:
                        att_dma(1)
                    if img == 1 and ci == 2:
                        seg_dma(1)
                    chunk_mult(img, ci)
                    chunk_abs(img, ci)
                    if img == 1 and ci == 3:
                        att3r_copy(1)
                    chunk_ut(img, ci, first=(ci == 0))
                    # defer A matmuls one chunk so PE never waits on ACT
                    if ci > 0:
                        chunk_a(img, ci - 1, first=(ci == 1))
                    if img == 1 and ci == 5:
                        epilogue(0)
                chunk_a(img, nchunks - 1, first=False)
                closers(img)
            epilogue(1)

    nc.compile()
    return nc


_NC_CACHE = None


def _get_nc():
    global _NC_CACHE
    if _NC_CACHE is None:
        _NC_CACHE = _build()
    return _NC_CACHE


def run(inputs: dict, trace: bool = False):
    """Run on 8 NeuronCores; returns (out [16,1,256,256], BassKernelResults)."""
    aff = np.ascontiguousarray(np.asarray(inputs["affinity"], dtype=np.float32))
    att = np.ascontiguousarray(np.asarray(inputs["attention"], dtype=np.float32))
    cs = np.ascontiguousarray(
        np.asarray(inputs["current_segmentation"], dtype=np.float32))
    co = np.ascontiguousarray(
        np.asarray(inputs["coarse_segmentation"], dtype=np.float32))
    band = _band_matrix()

    nc = _get_nc()
    in_maps = []
    for c in range(N_CORES):
        s = slice(c * B_CORE, (c + 1) * B_CORE)
        in_maps.append({
            "affinity": np.ascontiguousarray(aff[s]),
            "attention": np.ascontiguousarray(att[s]),
            "current_segmentation": np.ascontiguousarray(cs[s]),
            "coarse_segmentation": np.ascontiguousarray(co[s]),
            "band": band,
        })
    last_err = None
    for attempt in range(3):
        try:
            res = run_bass_kernel_spmd(nc, in_maps, list(range(N_CORES)),
                                       trace=trace)
            break
        except Exception as e:  # transient NRT_EXEC_UNIT_UNRECOVERABLE flakes
            last_err = e
            import time
            time.sleep(10)
    else:
        raise last_err
    full = np.concatenate([res.results[c]["out"] for c in range(N_CORES)], axis=0)
    return full, res


def kernel(**inputs) -> np.ndarray:
    out, _ = run(inputs, trace=False)
    return out
